# revision 20
# baseline (speedup 1.0000x reference)
"""Trainium2 Bass kernel for nn_ConvNode (tree-conv GNN message passing).

Math (per token t):
  m[c]   = (children[t,c] != 0)                       # valid-child mask
  ns     = sum_c m[c]
  cr[c]  = ns==1 ? 0.5*[c==0] : c*m[c]/(ns-1)
  cl[c]  = (1 - cr[c]) * m[c]
  mix_r  = sum_c cr[c] * cv[t,c,:]
  mix_l  = sum_c cl[c] * cv[t,c,:]
  out    = lrelu(nodes[t]@w_t + mix_r@w_r + mix_l@w_l + conv, 0.01)

Implementation: pure data-parallel over batch (4 batches / core, 8 cores).
Per core (2048 tokens):
  Stage 1 (PE): for each group of 8 tokens, a [128,128] block-diagonal
    coefficient matrix (stationary, float32r) multiplies the group's
    children_vectors [128=(8 tok x 16 child), 256] (streaming) producing
    [16,256] = (mix_r | mix_l) rows; 8 groups accumulate into one
    [128,256] PSUM tile via zero-padded disjoint output rows.
  Transpose (PE): mixed [tok,f] -> [f,tok] for use as stage-2 lhsT.
  Stage 2 (PE): psum[t,o] = nodesT.T@w_t + mixT_r.T@w_r + mixT_l.T@w_l
    + ones.T@conv, all float32r, then leaky-relu on DVE and DMA out.
Coefficients are computed on-device in a (child-slot, group) layout obtained
via an fp16 xbar DMA transpose of `children` through a DRAM scratch buffer.
"""

import sys

if "/opt/trn_rl_repo" not in sys.path:
    sys.path.insert(0, "/opt/trn_rl_repo")

import numpy as np

import concourse.bass as bass
import concourse.bacc as bacc
import concourse.tile as tile
from concourse import mybir

F32 = mybir.dt.float32
F32R = mybir.dt.float32r
F16 = mybir.dt.float16
I32 = mybir.dt.int32
Alu = mybir.AluOpType

B, T, C, F, O = 32, 512, 16, 256, 256
NCORES = 8
NTOK_FULL = (B // NCORES) * T  # 2048 tokens per core


def _r(ap):
    return ap.bitcast(F32R)


def build_bass(ntok=NTOK_FULL):
    assert ntok % 128 == 0 and (ntok // 64) % 2 == 0
    ngrp = ntok // 8      # token groups of 8
    nslab = ngrp // 8     # slabs of 8 groups (64 tokens)
    ntile = ntok // 128   # 128-token tiles

    nc = bacc.Bacc("TRN2", name="convnode")

    nodes_d = nc.dram_tensor("nodes", [ntok, F], F32, kind="ExternalInput")
    ch_d = nc.dram_tensor("children", [ntok, C], I32, kind="ExternalInput")
    cv_d = nc.dram_tensor("cv", [ntok * C, F], F32, kind="ExternalInput")
    wt_d = nc.dram_tensor("w_t", [F, O], F32, kind="ExternalInput")
    wr_d = nc.dram_tensor("w_r", [F, O], F32, kind="ExternalInput")
    wl_d = nc.dram_tensor("w_l", [F, O], F32, kind="ExternalInput")
    conv_d = nc.dram_tensor("conv", [1, O], F32, kind="ExternalInput")
    out_d = nc.dram_tensor("out", [ntok, O], F32, kind="ExternalOutput")

    with tile.TileContext(nc) as tc:
        _build(nc, tc, ntok, ngrp, nslab, ntile,
               nodes_d, ch_d, cv_d, wt_d, wr_d, wl_d, conv_d, out_d)
    nc.compile()
    return nc


def _build(nc, tc, ntok, ngrp, nslab, ntile,
           nodes_d, ch_d, cv_d, wt_d, wr_d, wl_d, conv_d, out_d):
    from contextlib import ExitStack

    ctx = ExitStack()
    with ctx:
        ones = ctx.enter_context(tc.tile_pool(name="ones", bufs=1))
        coefp = ctx.enter_context(tc.tile_pool(name="coefp", bufs=1))
        slabp = ctx.enter_context(tc.tile_pool(name="slabp", bufs=1))
        cvp = ctx.enter_context(tc.tile_pool(name="cvp", bufs=3))
        mixp = ctx.enter_context(tc.tile_pool(name="mixp", bufs=3))
        mixtp = ctx.enter_context(tc.tile_pool(name="mixtp", bufs=4))
        nodp = ctx.enter_context(tc.tile_pool(name="nodp", bufs=3))
        outp = ctx.enter_context(tc.tile_pool(name="outp", bufs=3))
        ps1 = ctx.enter_context(tc.tile_pool(name="ps1", bufs=2, space="PSUM"))
        pst = ctx.enter_context(tc.tile_pool(name="pst", bufs=2, space="PSUM"))
        ps3 = ctx.enter_context(tc.tile_pool(name="ps3", bufs=2, space="PSUM"))
        ps0 = ctx.enter_context(tc.tile_pool(name="ps0", bufs=1, space="PSUM"))

        # ---------------- one-time constants ----------------
        # E [128, 16]: E[k, q] = 1 iff k//16 == q%8   (block-diag placement mask)
        E = ones.tile([128, 16], F32, tag="E")
        nc.vector.memset(E, 1.0)
        nc.gpsimd.affine_select(out=E, in_=E, pattern=[[0, 2], [-16, 8]],
                                compare_op=Alu.is_ge, fill=0.0,
                                base=0, channel_multiplier=1)
        nc.gpsimd.affine_select(out=E, in_=E, pattern=[[0, 2], [16, 8]],
                                compare_op=Alu.is_ge, fill=0.0,
                                base=15, channel_multiplier=-1)
        # E8T [8, 128]: E8T[j, k] = 1 iff k//16 == j  (partition replicate)
        E8T = ones.tile([8, 128], F32, tag="E8T")
        nc.vector.memset(E8T, 1.0)
        nc.gpsimd.affine_select(out=E8T, in_=E8T, pattern=[[1, 8], [0, 16]],
                                compare_op=Alu.is_ge, fill=0.0,
                                base=0, channel_multiplier=-1)
        nc.gpsimd.affine_select(out=E8T, in_=E8T, pattern=[[-1, 8], [0, 16]],
                                compare_op=Alu.is_ge, fill=0.0,
                                base=0, channel_multiplier=1)
        # identity [128, 128] for PE transposes
        ident = ones.tile([128, 128], F32, tag="ident")
        nc.vector.memset(ident, 1.0)
        nc.gpsimd.affine_select(out=ident, in_=ident, pattern=[[1, 128]],
                                compare_op=Alu.is_ge, fill=0.0,
                                base=0, channel_multiplier=-1)
        nc.gpsimd.affine_select(out=ident, in_=ident, pattern=[[-1, 128]],
                                compare_op=Alu.is_ge, fill=0.0,
                                base=0, channel_multiplier=1)
        # cidx [128, 1] f32: k % 16 ; sel0h = 0.5*(cidx == 0)
        kk = ones.tile([128, 1], I32, tag="kk")
        nc.gpsimd.iota(kk, pattern=[[0, 1]], base=0, channel_multiplier=1)
        nc.vector.tensor_scalar(out=kk, in0=kk, scalar1=15, scalar2=None,
                                op0=Alu.bitwise_and)
        cidx = ones.tile([128, 1], F32, tag="cidx")
        nc.vector.tensor_copy(cidx, kk)
        sel0h = ones.tile([128, 1], F32, tag="sel0h")
        nc.vector.tensor_scalar(out=sel0h, in0=cidx, scalar1=0.0, scalar2=0.5,
                                op0=Alu.is_equal, op1=Alu.mult)
        # ones row for the bias matmul; conv row
        ones1f = ones.tile([1, 128], F32, tag="ones1f")
        nc.vector.memset(ones1f, 1.0)
        ones1 = ones.tile([1, 128], F32R, tag="ones1")
        nc.vector.tensor_copy(ones1, ones1f)
        zero1 = ones.tile([128, 1], F32, tag="zero1")
        nc.vector.memset(zero1, 0.0)
        conv_sb = ones.tile([1, O], F32R, tag="conv_sb")
        nc.sync.dma_start(out=conv_sb, in_=conv_d[:, :].bitcast(F32R))
        # weights: [2][128, 256] per path
        w_sb = {}
        for nm, wd in (("t", wt_d), ("r", wr_d), ("l", wl_d)):
            w = ones.tile([128, 2, O], F32R, tag=f"w_{nm}")
            nc.sync.dma_start(out=w, in_=wd[:, :].rearrange("(a p) o -> p a o", p=128)
                              .bitcast(F32R))
            w_sb[nm] = w

        # ---------------- coefficients (k=(j,c), g) layout ----------------
        # children [g, k] view -> f32 cast -> PE transpose -> chf [k, g]
        gcols = ngrp  # one column per 8-token group
        ch_gk = ch_d[:, :].rearrange("t c -> (t c)").rearrange("(g k) -> g k", k=128)
        chf = coefp.tile([128, gcols], F32, tag="chf")
        for blk in range((gcols + 127) // 128):
            rows = min(128, gcols - 128 * blk)
            ch_i = coefp.tile([128, 128], I32, tag="ch_i")
            nc.sync.dma_start(out=ch_i[:rows, :],
                              in_=ch_gk[128 * blk:128 * blk + rows, :])
            ch_f = coefp.tile([128, 128], F32, tag="ch_f")
            nc.vector.tensor_copy(ch_f[:rows, :], ch_i[:rows, :])
            ptc = pst.tile([128, 128], F32, tag="pt")
            nc.tensor.transpose(ptc[:, 0:rows], ch_f[:rows, :], ident[:rows, :rows])
            nc.scalar.copy(chf[:, 128 * blk:128 * blk + rows], ptc[:, 0:rows])
        # m = min(chf, 1)
        m_pt = coefp.tile([128, gcols], F32R, tag="m_pt")
        nc.vector.tensor_scalar_min(m_pt, chf, 1.0)
        E_r = coefp.tile([128, 8], F32R, tag="E_r")
        nc.vector.tensor_copy(E_r, E[:, 0:8])
        # ns rows [8, g] via E8 reduce matmul (E8 = E[:, 0:8])
        ps_ns = ps0.tile([8, gcols], F32, tag="ps_ns")
        nc.tensor.matmul(ps_ns, lhsT=E_r, rhs=m_pt,
                         start=True, stop=True)
        ns_sb = coefp.tile([8, gcols], F32, tag="ns_sb")
        nc.scalar.copy(ns_sb, ps_ns)
        # stats rows: [8, 2g]: cols 0:g = (ns==1), g:2g = 1/max(ns-1, .5)
        stats = coefp.tile([8, 2 * gcols], F32R, tag="stats")
        E8T_r = coefp.tile([8, 128], F32R, tag="E8T_r")
        nc.vector.tensor_copy(E8T_r, E8T)
        nc.vector.tensor_scalar(out=stats[:, 0:gcols], in0=ns_sb,
                                scalar1=1.0, scalar2=None, op0=Alu.is_equal)
        tmp8 = coefp.tile([8, gcols], F32, tag="tmp8")
        nc.vector.tensor_scalar(out=tmp8, in0=ns_sb, scalar1=-1.0, scalar2=0.5,
                                op0=Alu.add, op1=Alu.max)
        with nc.allow_low_precision(reason="float32r output feeds PE intentionally"):
            nc.vector.reciprocal(out=stats[:, gcols:2 * gcols], in_=tmp8)
        # replicate to all 16 slots of each token: [128, 2g]
        ps_rep = ps0.tile([128, 2 * gcols], F32, tag="ps_rep")
        nc.tensor.matmul(ps_rep, lhsT=E8T_r, rhs=stats,
                         start=True, stop=True)
        statspt = coefp.tile([128, 2 * gcols], F32, tag="statspt")
        nc.scalar.copy(statspt, ps_rep)
        is1 = statspt[:, 0:gcols]
        rinv = statspt[:, gcols:2 * gcols]
        # cr = (cidx*m)*rinv*(1-is1) + 0.5*sel0*is1 ; cl = m - cr*m
        cr = coefp.tile([128, gcols], F32, tag="cr")
        cl = coefp.tile([128, gcols], F32, tag="cl")
        tmp = coefp.tile([128, gcols], F32, tag="tmp")
        nc.vector.tensor_scalar_mul(tmp, m_pt, cidx)            # c*m
        nc.vector.tensor_mul(cr, tmp, rinv)                     # c*m/(ns-1)
        nc.vector.tensor_mul(tmp, cr, is1)
        nc.vector.tensor_sub(cr, cr, tmp)                       # *(1-is1)
        nc.vector.tensor_scalar_mul(tmp, is1, sel0h)            # 0.5*sel0*is1
        nc.vector.tensor_add(cr, cr, tmp)
        nc.vector.tensor_mul(tmp, cr, m_pt)                     # cr*m
        nc.vector.tensor_sub(cl, m_pt, tmp)                     # m - cr*m

        # ---------------- BD slabs (persistent, zero-init once) ----------------
        NSLOT = 4
        slabs = []
        zbc = bass.AP(tensor=zero1.tensor, offset=zero1.offset,
                      ap=[zero1.ap[0], [0, 8 * 128]])
        for s in range(NSLOT):
            sl = slabp.tile([128, 8 * 128], F32R, tag=f"slab{s}")
            nc.vector.tensor_copy(sl, zbc)
            slabs.append(sl)

        def fill_slab(sl, slab_idx):
            # block gl spans cols [128*gl, 128*gl+128); within it the r-band
            # sits at cols 8*gl + (0..8) (-> psum rows 0..63, token order) and
            # the l-band at 64 + 8*gl + (0..8) (-> psum rows 64..127).
            g0 = slab_idx * 8
            for path, coef in ((0, cr), (1, cl)):
                out_ap = bass.AP(tensor=sl.tensor, offset=sl.offset + 64 * path,
                                 ap=[sl.ap[0], [136, 8], [1, 8]])
                in0 = bass.AP(tensor=coef.tensor, offset=coef.offset + g0,
                              ap=[coef.ap[0], [1, 8], [0, 8]])
                in1 = bass.AP(tensor=E.tensor, offset=E.offset,
                              ap=[E.ap[0], [0, 8], [1, 8]])
                nc.vector.tensor_mul(out_ap, in0, in1)

        # ---------------- main loop ----------------
        cv_v = cv_d[:, :].rearrange("(s p) f -> s p f", p=128)  # per-group rows
        nodes_v = nodes_d[:, :].rearrange("(i p) f -> i p f", p=128)
        out_v = out_d[:, :].rearrange("(i p) o -> i p o", p=128)

        mixt_tiles = [None]  # lhsT tile of the current 128-token tile

        for s in range(nslab):
            sl = slabs[s % NSLOT]
            fill_slab(sl, s)
            # cv for 64 tokens = 8 groups: [128, 8*256]
            cv64 = cvp.tile([128, 8, F], F32R, tag="cv64")
            nc.sync.dma_start(out=cv64, in_=cv_v[8 * s:8 * s + 8, :, :]
                              .rearrange("g p f -> p g f").bitcast(F32R))
            pm = ps1.tile([128, F], F32, tag="pm")
            for gl in range(8):
                nc.tensor.matmul(pm, lhsT=sl[:, 128 * gl:128 * gl + 128],
                                 rhs=cv64[:, gl, :],
                                 start=(gl == 0), stop=(gl == 7))
            mixed = mixp.tile([128, F], F32, tag="mixed")
            nc.scalar.copy(mixed, pm)
            # transpose both f-halves; scatter r/l 64-col halves into the
            # 128-token tile's lhsT layout [128, h, path, tok]
            if s % 2 == 0:
                mixt_tiles[0] = mixtp.tile([128, 2, 2, 128], F32R, tag="mixt", name="mixt")
            mixt = mixt_tiles[0]
            half = s % 2
            for h in range(2):
                pt = pst.tile([128, 128], F32, tag="pt")
                nc.tensor.transpose(pt, mixed[:, 128 * h:128 * h + 128], ident)
                for path in range(2):
                    nc.scalar.copy(mixt[:, h, path, 64 * half:64 * half + 64],
                                   pt[:, 64 * path:64 * path + 64])

            if s % 2 == 1:
                i = s // 2  # 128-token tile index
                # nodes tile + transpose
                nod = nodp.tile([128, F], F32, tag="nod")
                nc.sync.dma_start(out=nod, in_=nodes_v[i, :, :])
                nodt = nodp.tile([128, 2, 128], F32R, tag="nodt")
                for h in range(2):
                    pt = pst.tile([128, 128], F32, tag="pt")
                    nc.tensor.transpose(pt, nod[:, 128 * h:128 * h + 128], ident)
                    nc.scalar.copy(nodt[:, h, :], pt)
                po = ps3.tile([128, O], F32, tag="po")
                first = True
                for h in range(2):
                    nc.tensor.matmul(po, lhsT=nodt[:, h, :],
                                     rhs=w_sb["t"][:, h, :],
                                     start=first, stop=False)
                    first = False
                for path, nm in ((0, "r"), (1, "l")):
                    for h in range(2):
                        nc.tensor.matmul(po, lhsT=mixt[:, h, path, :],
                                         rhs=w_sb[nm][:, h, :],
                                         start=False, stop=False)
                nc.tensor.matmul(po, lhsT=ones1, rhs=conv_sb,
                                 start=False, stop=True)
                # bias is in po via conv matmul; now lrelu: max(x, 0.01x)
                ot = outp.tile([128, O], F32, tag="ot")
                nc.scalar.copy(ot, po)
                ot2 = outp.tile([128, O], F32, tag="ot2")
                nc.vector.scalar_tensor_tensor(
                    out=ot2, in0=ot, scalar=0.01, in1=ot,
                    op0=Alu.mult, op1=Alu.max)
                nc.sync.dma_start(out=out_v[i, :, :], in_=ot2)


_NC_CACHE = {}
LAST_EXEC_NS = None


def _get_nc(ntok):
    if ntok not in _NC_CACHE:
        _NC_CACHE[ntok] = build_bass(ntok)
    return _NC_CACHE[ntok]


def kernel(nodes, children_vectors, w_t, w_l, w_r, conv, children):
    global LAST_EXEC_NS
    from concourse import bass_utils

    nc = _get_nc(NTOK_FULL)
    bpc = B // NCORES
    in_maps = []
    for i in range(NCORES):
        sl = slice(i * bpc, (i + 1) * bpc)
        in_maps.append({
            "nodes": np.ascontiguousarray(nodes[sl]).reshape(NTOK_FULL, F),
            "children": np.ascontiguousarray(children[sl]).reshape(NTOK_FULL, C),
            "cv": np.ascontiguousarray(children_vectors[sl]).reshape(NTOK_FULL * C, F),
            "w_t": np.ascontiguousarray(w_t),
            "w_r": np.ascontiguousarray(w_r),
            "w_l": np.ascontiguousarray(w_l),
            "conv": np.ascontiguousarray(conv).reshape(1, O),
        })
    res = bass_utils.run_bass_kernel_spmd(nc, in_maps, core_ids=list(range(NCORES)))
    if res.exec_time_ns is not None:
        LAST_EXEC_NS = res.exec_time_ns
    out = np.concatenate([r["out"].reshape(bpc, T, O) for r in res.results], axis=0)
    return out
